# revision 39
# baseline (speedup 1.0000x reference)
"""Causal self-attention on 8 TRN2 NeuronCores.

Sharding: core c = (batch b = c//2, head-group g = c%2).  Each core computes
the full attention for one batch and 8 of the 16 heads (column-sharded
Wq/Wk/Wv, row-sharded Wproj), producing a partial output projection; the two
partials per batch are summed on the host (the row-parallel all-reduce).

Matmul operands are bf16 (fp32 psum accumulation).  Per-core dataflow:
  xT[c_in, t]  (host pre-transposed, bf16)
  qT/kT[cq, t] = Wq/Wk^T @ xT          (pair-packed: 2 heads per 128-part tile)
  v[t, cv]     = x @ Wv                (stored [t, head, 64+1] with a ones
                                        column so AV also yields the exp-sums)
  scoresT[k,q] = k @ qT  per head      (row-group-packed pair matmuls, K=64;
                                        both heads of a pair write one 2-bank
                                        psum tile so ONE ScalarE exp covers
                                        them)
  expT         = exp(0.125*scoresT)    (one activation per k-tile; diagonal
                                        blocks masked post-exp by a 0/1
                                        triangle multiply on GpSimd, both
                                        heads in one strided op)
  outT[dv|s,q] = [v|1].T @ expT        (psum accumulate over k tiles; row 64 =
                                        softmax sums)
  outT_scaled  = outT[0:64] * (1/outT[64])  (sums broadcast via K=1 ones
                                        outer-product matmul, DVE reciprocal
                                        + scale)
  y_partial    = outT_scaled.T @ Wproj_rows

The attention inner loop keeps TensorE saturated: QKV / output projection
matmul groups are generators "pumped" between attention steps, ordered
per-pair so every projection completes well before its pair starts (no flush
stalls).  Score tiles live in a dedicated double-buffered psum pool isolated
from the filler pool, so pool-rotation WAR waits never block the score
matmuls.  The attention-output psum is single-buffered; each pair's tail
evacuates it with two DVE copies at the next pair's first step, two steps
before that pair's first AV matmul needs the banks.
"""

import numpy as np
import ml_dtypes
from contextlib import ExitStack

import concourse.tile as tile
from concourse import bacc, mybir
from concourse.bass import ts
from concourse.bass_utils import run_bass_kernel_spmd

F32 = mybir.dt.float32
BF16 = mybir.dt.bfloat16
AF = mybir.ActivationFunctionType

N_CORES = 8
T = 1024
C = 1024
D = 64          # head dim
HL = 8          # heads per core
CL = HL * D     # 512 local channels
NKT = 8         # k (key) tiles of 128
NPAIR = 4       # head pairs per core

_CACHE = {}


def _build():
    nc = bacc.Bacc("TRN2", target_bir_lowering=False, debug=False,
                   num_devices=N_CORES)
    xt = nc.dram_tensor("xt", [C, T], BF16, kind="ExternalInput").ap()
    wq = nc.dram_tensor("wq", [C, CL], BF16, kind="ExternalInput").ap()
    wk = nc.dram_tensor("wk", [C, CL], BF16, kind="ExternalInput").ap()
    wv = nc.dram_tensor("wv", [C, CL], BF16, kind="ExternalInput").ap()
    wp = nc.dram_tensor("wp", [CL, C], BF16, kind="ExternalInput").ap()
    # 0/1 keep-mask: tri[p, f] = 1 where f >= p  (duplicated for both heads)
    tri = nc.dram_tensor("tri", [128, 256], BF16, kind="ExternalInput").ap()
    ones_a = nc.dram_tensor("ones_a", [1, 64], BF16, kind="ExternalInput").ap()
    ones_b = nc.dram_tensor("ones_b", [128, 8], BF16, kind="ExternalInput").ap()
    y = nc.dram_tensor("y", [T, C], BF16, kind="ExternalOutput").ap()

    xt_r = xt.rearrange("(kt p) t -> kt p t", p=128)
    w_r = {n: w.rearrange("(kt p) n -> kt p n", p=128)
           for n, w in (("wq", wq), ("wk", wk), ("wv", wv))}

    with tile.TileContext(nc) as tc, ExitStack() as ctx:
        const = ctx.enter_context(tc.tile_pool(name="const", bufs=1))
        big = ctx.enter_context(tc.tile_pool(name="big", bufs=1))
        ps_main = ctx.enter_context(
            tc.tile_pool(name="ps_main", bufs=2, space="PSUM"))
        ps_score = ctx.enter_context(
            tc.tile_pool(name="ps_score", bufs=2, space="PSUM"))
        ps_out = ctx.enter_context(
            tc.tile_pool(name="ps_out", bufs=1, space="PSUM"))
        sb_exp = ctx.enter_context(tc.tile_pool(name="sb_exp", bufs=4))
        sb_tmp = ctx.enter_context(tc.tile_pool(name="sb_tmp", bufs=2))
        sb_y = ctx.enter_context(tc.tile_pool(name="sb_y", bufs=4))

        # ---- load phase ----
        # The first attention pair needs only the pair-0 (128-col) slices of
        # Wq/Wk, so those load FIRST (0.25MB each) ahead of the xt halves on
        # the two HWDGE queues; the pair-1..3 remainders trail (wqR on
        # scalar, wkR on the gpsimd SWDGE queue), wv/wp behind xt on sync.
        tri_sb = const.tile([128, 2, 128], BF16)
        nc.sync.dma_start(
            out=tri_sb[:], in_=tri.rearrange("p (b f) -> p b f", b=2))
        ones_a_sb = const.tile([65, 64], BF16)
        nc.sync.dma_start(out=ones_a_sb[64:65, :], in_=ones_a)
        ones_b_sb = const.tile([128, 8], BF16)
        nc.sync.dma_start(out=ones_b_sb[:], in_=ones_b)
        xt_sb, wv_sb = [], []
        wq0_sb, wqR_sb, wk0_sb, wkR_sb = [], [], [], []
        for kt in range(NKT):
            wc = big.tile([128, 128], BF16, name=f"wq0_{kt}")
            nc.scalar.dma_start(out=wc[:], in_=w_r["wq"][kt][:, 0:128])
            wq0_sb.append(wc)
            wc = big.tile([128, 128], BF16, name=f"wk0_{kt}")
            nc.sync.dma_start(out=wc[:], in_=w_r["wk"][kt][:, 0:128])
            wk0_sb.append(wc)
        for kt in range(NKT):
            xc = big.tile([128, T], BF16, name=f"xt{kt}")
            (nc.scalar if kt < 4 else nc.sync).dma_start(
                out=xc[:], in_=xt_r[kt])
            xt_sb.append(xc)
        for kt in range(NKT):
            wc = big.tile([128, 384], BF16, name=f"wqR{kt}")
            nc.scalar.dma_start(out=wc[:], in_=w_r["wq"][kt][:, 128:512])
            wqR_sb.append(wc)
            wc = big.tile([128, 384], BF16, name=f"wkR{kt}")
            nc.gpsimd.dma_start(out=wc[:], in_=w_r["wk"][kt][:, 128:512])
            wkR_sb.append(wc)
        for kt in range(NKT):
            wc = big.tile([128, CL], BF16, name=f"wv{kt}")
            nc.sync.dma_start(out=wc[:], in_=w_r["wv"][kt])
            wv_sb.append(wc)
        wp_sb = big.tile([128, NPAIR, C], BF16)
        nc.sync.dma_start(
            out=wp_sb[:], in_=wp.rearrange("(r p) n -> p r n", p=128))

        qT_sb = big.tile([128, NPAIR, T], BF16)
        kT_sb = big.tile([128, NPAIR, T], BF16)
        v_sb = big.tile([128, NKT, HL, D + 1], BF16)
        projT_sb = big.tile([128, NPAIR, T], BF16)

        # ---- PE work generators ----
        def qkv_group(dst, w0_sb, wR_sb, m, nt):
            ps = ps_main.tile([128, 512], F32, name="ps")
            for kt in range(NKT):
                w = (w0_sb[kt][:, :] if m == 0
                     else wR_sb[kt][:, ts(m - 1, 128)])
                nc.tensor.matmul(
                    ps[:], w, xt_sb[kt][:, ts(nt, 512)],
                    start=(kt == 0), stop=(kt == NKT - 1))
                if kt % 2 == 1:
                    yield
            nc.vector.tensor_copy(dst[:, m, ts(nt, 512)], ps[:])

        def v_group(tt):
            ps = ps_main.tile([128, 512], F32, name="ps")
            for kt in range(NKT):
                nc.tensor.matmul(
                    ps[:], xt_sb[kt][:, ts(tt, 128)], wv_sb[kt][:],
                    start=(kt == 0), stop=(kt == NKT - 1))
                if kt % 2 == 1:
                    yield
            nc.vector.tensor_copy(
                v_sb[:, tt, :, 0:D],
                ps[:].rearrange("p (h d) -> p h d", h=HL))
            nc.vector.tensor_copy(v_sb[:, tt, :, D], ones_b_sb[:])

        def proj_group(q0, tt2, n2):
            ps = ps_main.tile([128, 512], F32, name="ps")
            for r in range(NPAIR):
                nc.tensor.matmul(
                    ps[:],
                    projT_sb[:, r, q0 + 128 * tt2:q0 + 128 * (tt2 + 1)],
                    wp_sb[:, r, ts(n2, 512)],
                    start=(r == 0), stop=(r == NPAIR - 1))
                if r % 2 == 1:
                    yield
            yt = sb_y.tile([128, 512], BF16)
            nc.vector.tensor_copy(yt[:], ps[:])
            # bf16 output chunks alternate between the two HWDGE queues so
            # the drain's output traffic clears in half the time.
            (nc.sync if (tt2 + n2) % 2 == 0 else nc.scalar).dma_start(
                out=y[q0 + 128 * tt2:q0 + 128 * (tt2 + 1), ts(n2, 512)],
                in_=yt[:])

        fillers = []  # [tag, generator]

        def pump(n):
            while n > 0 and fillers:
                tag, g = fillers[0]
                try:
                    next(g)
                    n -= 1
                except StopIteration:
                    fillers.pop(0)

        def pump_flush_group():
            # run the in-flight filler group to completion so its ps_main
            # slot is released before the tail's bc tiles rotate onto it
            # (an open group's psum held across the bc allocations would
            # deadlock the PE queue).
            if fillers:
                tag, g = fillers[0]
                for _ in g:
                    pass
                fillers.pop(0)

        def flush(tags):
            i = 0
            while i < len(fillers):
                tag, g = fillers[i]
                if tag in tags:
                    for _ in g:
                        pass
                    fillers.pop(i)
                else:
                    i += 1

        # ---- QKV upfront: only what the first attention pair (q-half 0,
        # pair 0) needs to START: k/q pair-0 keys+queries 0..511 (their
        # 128-col weight slices arrive first on the DMA queues).  v0..3 lead
        # the filler queue -- the first AV (delayed to step 2) picks them up
        # as wv lands.  q-half 0 runs first (it only needs the n0 halves and
        # v0..3), then q-half 1 whose extra inputs (v4..7, the n1 halves)
        # and the q-half-0 projection stream in as fillers, keeping the PE
        # dense to the end.
        for _ in qkv_group(kT_sb, wk0_sb, wkR_sb, 0, 0):
            pass
        for _ in qkv_group(qT_sb, wq0_sb, wqR_sb, 0, 0):
            pass
        for tt in range(4):
            fillers.append((f"v{tt}", v_group(tt)))
        for m in range(1, NPAIR):
            fillers.append((f"k{m}n0", qkv_group(kT_sb, wk0_sb, wkR_sb, m, 0)))
            fillers.append((f"q{m}n0", qkv_group(qT_sb, wq0_sb, wqR_sb, m, 0)))
        for tt in range(4, NKT):
            fillers.append((f"v{tt}", v_group(tt)))
        fillers.append(("k0n1", qkv_group(kT_sb, wk0_sb, wkR_sb, 0, 1)))
        fillers.append(("q0n1", qkv_group(qT_sb, wq0_sb, wqR_sb, 0, 1)))
        for m in range(1, NPAIR):
            fillers.append((f"k{m}n1", qkv_group(kT_sb, wk0_sb, wkR_sb, m, 1)))
            fillers.append((f"q{m}n1", qkv_group(qT_sb, wq0_sb, wqR_sb, m, 1)))

        # ---- attention ----
        # outAB rows: 0..63 = head dims, 64 = exp-sums (ones column of v).
        # Two-phase tail: `evac` frees the single-buffered outAB with two
        # DVE copies at the next pair's step 0; `norm` (step 1) broadcasts
        # the sums row via a K=1 ones outer-product matmul into psum,
        # reciprocates, and scales into projT's packed layout.
        def make_tail(m, outAB, q0):
            state = {}

            def evac():
                rr = sb_tmp.tile([65, 2, 512], BF16, name="rr")
                nc.vector.tensor_copy(rr[64:65, :, :], outAB[64:65, :, :])
                oc = sb_tmp.tile([64, 2, 512], F32, name="oc")
                nc.vector.tensor_copy(oc[:, :, :], outAB[0:64, :, :])
                state["rr"], state["oc"] = rr, oc

            def norm():
                rr, oc = state["rr"], state["oc"]
                pump_flush_group()
                bcs = []
                for hh in range(2):
                    bc = ps_main.tile([64, 512], F32, name="ps")
                    nc.tensor.matmul(
                        bc[:], ones_a_sb[64:65, :], rr[64:65, hh, :],
                        start=True, stop=True, tile_position=(64, 0))
                    bcs.append(bc)
                bcr = sb_tmp.tile([64, 2, 512], F32, name="bcr")
                for hh in range(2):
                    nc.vector.reciprocal_approx_fast(
                        out=bcr[:, hh, :], in_=bcs[hh][:])
                pump(2)
                nc.vector.tensor_mul(
                    projT_sb[0:64, m, q0:q0 + 512], oc[:, 0, :], bcr[:, 0, :])
                tb = sb_tmp.tile([64, 512], BF16, name="tb")
                nc.vector.tensor_mul(tb[:], oc[:, 1, :], bcr[:, 1, :])
                nc.sync.dma_start(
                    out=projT_sb[64:128, m, q0:q0 + 512], in_=tb[:])

            return [evac, norm]

        for qt in (0, 1):
            q0 = 512 * qt
            pend_tail = []
            for m in range(NPAIR):
                need = {f"q{m}n{qt}", f"k{m}n0", f"k{m}n{qt}"}
                if qt == 1 and m == 0:
                    need |= {"v4", "v5", "v6", "v7"}
                flush(need)
                kts = list(range(4 * qt + 4))
                outAB = ps_out.tile([65, 2, 512], F32)
                pend = []
                for i in list(range(len(kts))) + [None, None]:
                    if i is not None:
                        kt = kts[i]
                        off = max(0, 128 * kt - q0)
                        qcols = slice(q0 + off, q0 + 512)
                        sp = ps_score.tile([128, 2, 512], F32, name="sp")
                        for hh, po in ((0, 0), (1, 64)):
                            nc.tensor.matmul(
                                sp[:, hh, off:512],
                                kT_sb[po:po + 64, m, ts(kt, 128)],
                                qT_sb[po:po + 64, m, qcols],
                                start=True, stop=True,
                                tile_position=(po, 0))
                        e4 = sb_exp.tile([128, 2, 512], BF16, name="et")
                        nc.scalar.activation(
                            e4[:, :, off:512], sp[:, :, off:512],
                            AF.Exp, scale=0.125)
                        if kt >= 4 * qt:  # diagonal: zero upper triangle
                            nc.gpsimd.tensor_mul(
                                e4[:, :, off:off + 128],
                                e4[:, :, off:off + 128], tri_sb[:, :, :])
                        pend.append((e4, i, off))
                    # previous pair's tail: evacuate outAB at step 0 (two
                    # steps before this pair's first AV needs the banks),
                    # normalize at step 1.
                    if pend_tail and (i in (0, 1) or i is None):
                        pend_tail.pop(0)()
                    pump(2)
                    if len(pend) > 2 or (i is None and pend):
                        (e4p, pi, poff) = pend.pop(0)
                        # the v group feeding this AV must be fully issued
                        # first, or the AV's read would predate the v write
                        # in program order (no-op once the group is done).
                        flush({f"v{kts[pi]}"})
                        for hh in range(2):
                            nc.tensor.matmul(
                                outAB[0:65, hh, poff:512],
                                v_sb[:, kts[pi], 2 * m + hh, :],
                                e4p[:, hh, poff:512],
                                start=(pi == 0), stop=(pi == len(kts) - 1))
                pend_tail = make_tail(m, outAB, q0)
            for fn in pend_tail:
                fn()
            pend_tail = []
            # queue this q-half's projection as PE filler for the next phase
            for tt2 in range(4):
                for n2 in range(2):
                    fillers.append((f"p{qt}", proj_group(q0, tt2, n2)))
        # drain remaining projection work
        while fillers:
            pump(len(fillers) * 8)

    nc.compile()
    return nc


def _program():
    if "nc" not in _CACHE:
        _CACHE["nc"] = _build()
    return _CACHE["nc"]


def _bf(a):
    return np.ascontiguousarray(a).astype(ml_dtypes.bfloat16)


def _in_maps(x, Wq, Wk, Wv, Wproj):
    tri1 = np.triu(np.ones((128, 128), dtype=np.float32))  # tri[p,f]=1, f>=p
    tri = np.ascontiguousarray(np.concatenate([tri1, tri1], axis=1)
                               ).astype(ml_dtypes.bfloat16)
    ones_a = np.ones((1, 64), dtype=ml_dtypes.bfloat16)
    ones_b = np.ones((128, 8), dtype=ml_dtypes.bfloat16)
    maps = []
    for c in range(N_CORES):
        b, g = c // 2, c % 2
        sl = slice(CL * g, CL * (g + 1))
        maps.append({
            "xt": _bf(x[b].T),
            "wq": _bf(Wq[:, sl]),
            "wk": _bf(Wk[:, sl]),
            "wv": _bf(Wv[:, sl]),
            "wp": _bf(Wproj[sl, :]),
            "tri": tri,
            "ones_a": ones_a,
            "ones_b": ones_b,
        })
    return maps


def run(x, Wq, Wk, Wv, Wproj, trace=False, **kwargs):
    nc = _program()
    maps = _in_maps(np.asarray(x, dtype=np.float32),
                    np.asarray(Wq, dtype=np.float32),
                    np.asarray(Wk, dtype=np.float32),
                    np.asarray(Wv, dtype=np.float32),
                    np.asarray(Wproj, dtype=np.float32))
    res = run_bass_kernel_spmd(nc, maps, core_ids=list(range(N_CORES)),
                               trace=trace, **kwargs)
    B = 4
    out = np.empty((B, T, C), dtype=np.float32)
    for b in range(B):
        out[b] = (res.results[2 * b]["y"].astype(np.float32)
                  + res.results[2 * b + 1]["y"].astype(np.float32))
    return out, res


def kernel(x, Wq, Wk, Wv, Wproj):
    out, _ = run(x, Wq, Wk, Wv, Wproj)
    return out
